# revision 1
# baseline (speedup 1.0000x reference)
"""Distributed Trainium2 Bass kernel for nn_CausalSelfAttention_66984309948568.

Strategy (8 NeuronCores, tensor-parallel over heads):
  - core h owns head h (8 heads, head_dim 128).
  - host pre-transposes x -> x^T and slices per-head weights; all matmul
    operands are float32r (FP22 read path, full PE rate at N=512).
  - per core: qkv projection in [d, t] layout; RMS-norm factors via
    ones-matmul column sums + exp(-0.5*ln(mean+eps)); RoPE applied with
    host-built cos/sin row matrices; causal attention streamed per
    512-column t-chunk in S^T = [s, t] layout (no max-subtraction needed:
    |scores| <= 15.4); softmax denominators from a ones-column matmul;
    y normalized via broadcast-reciprocal matmul; AllToAll exchanges
    per-head y slices so each core projects its own 512 rows of output.
  - host concatenates the 8 [512, 1024] slices.
"""

import sys

sys.path.insert(0, "/opt/trn_rl_repo")

import numpy as np
import concourse.bass as bass
import concourse.bacc as bacc
import concourse.mybir as mybir
from concourse import tile
from concourse.bass_utils import run_bass_kernel_spmd

N_CORES = 8
B, T, DIM = 1, 4096, 1024
NUM_HEADS, HEAD_DIM = 8, 128
HDIM = NUM_HEADS * HEAD_DIM
SCALE = 0.12
EPS = 1.1920928955078125e-07
NCHUNK = T // 512          # 8 t-chunks of 512
NTT = T // 128             # 32 t-tiles of 128
TSLICE = T // N_CORES      # 512 output rows per core

f32 = mybir.dt.float32
f32r = mybir.dt.float32r
FN = mybir.ActivationFunctionType
ALU = mybir.AluOpType
MASK_NEG = -30000.0


def _trunc22(a):
    b = np.ascontiguousarray(a, dtype=np.float32).copy()
    b.view(np.uint32)[...] &= 0xFFFFFC00
    return b


def _register_const(nc, value, dtype=f32):
    if (dtype, value) in nc.const_aps.aps:
        return
    t = nc.alloc_sbuf_tensor(f"const-{dtype.name}-{value}", [128, 1], dtype)
    nc.gpsimd.memset(t.ap(), value)
    nc.const_aps.aps[(dtype, value)] = t.ap()


def _build_program(repeat=1):
    nc = bacc.Bacc(num_devices=N_CORES)
    _register_const(nc, EPS)
    _register_const(nc, float(np.log(SCALE)))
    nc.all_engine_barrier()

    # ---- DRAM parameters (per-core values supplied via in_maps) ----
    xt_d = nc.declare_dram_parameter("xt", [DIM, T], f32r, isOutput=False)
    wq_d = nc.declare_dram_parameter("wq", [128, DIM], f32r, isOutput=False)
    wk_d = nc.declare_dram_parameter("wk", [128, DIM], f32r, isOutput=False)
    wv_d = nc.declare_dram_parameter("wv", [128, DIM], f32r, isOutput=False)
    vew_d = nc.declare_dram_parameter("vew", [128, T], f32, isOutput=False)
    cmat_d = nc.declare_dram_parameter("cmat", [128, T], f32, isOutput=False)
    smat_d = nc.declare_dram_parameter("smat", [128, T], f32, isOutput=False)
    mask_d = nc.declare_dram_parameter("maskc", [128, 2048], f32, isOutput=False)
    pw_d = nc.declare_dram_parameter("pw", [128, 8 * DIM], f32r, isOutput=False)
    onc_d = nc.declare_dram_parameter("ones_col", [128, 1], f32r, isOutput=False)
    onr_d = nc.declare_dram_parameter("ones_row", [1, 128], f32r, isOutput=False)
    id_d = nc.declare_dram_parameter("ident", [128, 128], f32, isOutput=False)
    out_d = nc.declare_dram_parameter("out", [TSLICE, DIM], f32, isOutput=True)

    ln_scale_q = float(np.log(SCALE))

    with tile.TileContext(nc, num_cores=N_CORES) as tc:
        with (
            tc.tile_pool(name="persist", bufs=1) as persist,
            tc.tile_pool(name="dram", bufs=1, space="DRAM") as dram,
        ):
            # persistent SBUF tensors
            qnT = persist.tile([128, T], f32r, tag="qnT")     # 0.12 * rope(norm(q))^T
            knT = persist.tile([128, T], f32r, tag="knT")     # rope(norm(k))^T
            v_sb = persist.tile([128, NTT * 128], f32r, tag="v_sb")  # v in [t,d] tiles
            maskc = persist.tile([128, 2048], f32, tag="maskc")
            onc = persist.tile([128, 1], f32r, tag="onc")
            onr = persist.tile([1, 128], f32r, tag="onr")
            ident = persist.tile([128, 128], f32, tag="ident")

            nc.gpsimd.dma_start(maskc[:], mask_d[:])
            nc.gpsimd.dma_start(onc[:], onc_d[:])
            nc.gpsimd.dma_start(onr[:], onr_d[:])
            nc.gpsimd.dma_start(ident[:], id_d[:])

            a2a_in = dram.tile([N_CORES * 128, TSLICE], f32r, tag="a2a_in")
            a2a_out = dram.tile([N_CORES * 128, TSLICE], f32r, tag="a2a_out")

            for _rep in range(repeat):
              # ================ Phase 1+2: qkv, norm, rope, v assembly ==========
              with (
                  tc.tile_pool(name="wpool", bufs=1) as wpool,
                  tc.tile_pool(name="ropec", bufs=1) as ropec,
                  tc.tile_pool(name="xt", bufs=2) as xt_pool,
                  tc.tile_pool(name="qkv_ps", bufs=1, space=bass.MemorySpace.PSUM) as qkv_ps,
                  tc.tile_pool(name="row_ps", bufs=1, space=bass.MemorySpace.PSUM) as row_ps,
                  tc.tile_pool(name="bc_ps", bufs=2, space=bass.MemorySpace.PSUM) as bc_ps,
                  tc.tile_pool(name="tr_ps", bufs=2, space=bass.MemorySpace.PSUM) as tr_ps,
                  tc.tile_pool(name="evac", bufs=3) as evac,
                  tc.tile_pool(name="rows", bufs=2) as rows,
                  tc.tile_pool(name="tmps", bufs=3) as tmps,
              ):
                  wq = wpool.tile([128, DIM], f32r, tag="wq")
                  wk = wpool.tile([128, DIM], f32r, tag="wk")
                  wv = wpool.tile([128, DIM], f32r, tag="wv")
                  cmat = ropec.tile([128, T], f32, tag="cmat")
                  smat = ropec.tile([128, T], f32, tag="smat")
                  vew = ropec.tile([128, T], f32, tag="vew")
                  nc.gpsimd.dma_start(wq[:], wq_d[:])
                  nc.gpsimd.dma_start(wk[:], wk_d[:])
                  nc.gpsimd.dma_start(wv[:], wv_d[:])
                  nc.gpsimd.dma_start(cmat[:], cmat_d[:])
                  nc.gpsimd.dma_start(smat[:], smat_d[:])
                  nc.gpsimd.dma_start(vew[:], vew_d[:])

                  for c in range(NCHUNK):
                      cs = bass.ts(c, 512)
                      ps_q = qkv_ps.tile([128, 512], f32, tag="ps_q")
                      ps_k = qkv_ps.tile([128, 512], f32, tag="ps_k")
                      ps_v = qkv_ps.tile([128, 512], f32, tag="ps_v")
                      # one 2MB DMA per chunk: [p, dt, col] <- xt[128*dt + p, 512c + col]
                      xt_t = xt_pool.tile([128, 8, 512], f32r, tag="xt")
                      nc.sync.dma_start(
                          xt_t[:],
                          xt_d[:, 512 * c : 512 * (c + 1)].rearrange(
                              "(dt p) col -> p dt col", p=128
                          ),
                      )
                      for dt in range(8):
                          st, sp = dt == 0, dt == 7
                          nc.tensor.matmul(ps_q[:], wq[:, bass.ts(dt, 128)], xt_t[:, dt, :], start=st, stop=sp)
                          nc.tensor.matmul(ps_k[:], wk[:, bass.ts(dt, 128)], xt_t[:, dt, :], start=st, stop=sp)
                          nc.tensor.matmul(ps_v[:], wv[:, bass.ts(dt, 128)], xt_t[:, dt, :], start=st, stop=sp)

                      # ---- v: transpose [d,t]->[t,d] per 128-tile, add ve ----
                      vTc = evac.tile([128, 512], f32, tag="vTc")
                      nc.vector.tensor_copy(vTc[:], ps_v[:])
                      for j in range(4):
                          i = 4 * c + j
                          ps_t = tr_ps.tile([128, 128], f32, tag="ps_t")
                          nc.tensor.transpose(ps_t[:], vTc[:, bass.ts(j, 128)], ident[:])
                          nc.vector.tensor_tensor(
                              v_sb[:, bass.ts(i, 128)], ps_t[:], vew[:, bass.ts(i, 128)], ALU.add
                          )

                      # ---- q, k: norm + rope ----
                      for which, ps_x, dstT in (("q", ps_q, qnT), ("k", ps_k, knT)):
                          xTc = evac.tile([128, 512], f32, tag="xTc")
                          nc.scalar.copy(xTc[:], ps_x[:])
                          sqc = tmps.tile([128, 512], f32r, tag="sqc")
                          nc.vector.tensor_tensor(sqc[:], xTc[:], xTc[:], ALU.mult)
                          ps_row = row_ps.tile([1, 512], f32, tag="ps_row")
                          nc.tensor.matmul(ps_row[:], onc[:], sqc[:], start=True, stop=True)
                          # rsq = exp(-0.5 * ln(mean + eps)) [* SCALE for q]
                          lnr = rows.tile([1, 512], f32, tag="lnr")
                          nc.scalar.activation(lnr[:], ps_row[:], FN.Ln, bias=EPS, scale=1.0 / HEAD_DIM)
                          rsq = rows.tile([1, 512], f32r, tag="rsq")
                          nc.scalar.activation(
                              rsq[:], lnr[:], FN.Exp,
                              bias=(ln_scale_q if which == "q" else 0.0), scale=-0.5,
                          )
                          ps_b = bc_ps.tile([128, 512], f32, tag="ps_b")
                          nc.tensor.matmul(ps_b[:], onr[:], rsq[:], start=True, stop=True)
                          # normalized tensor (f32 tmp), then rope into dstT (f32r)
                          tn = tmps.tile([128, 512], f32, tag="tn")
                          nc.vector.tensor_tensor(tn[:], xTc[:], ps_b[:], ALU.mult)
                          nc.vector.tensor_tensor(dstT[:, cs], tn[:], cmat[:, cs], ALU.mult)
                          # cross terms: smat[64:96]=+sinT (x2 -> y1 rows 0:32),
                          # smat[0:32]=-sinT (x1 -> y2 rows 64:96)
                          ut = tmps.tile([128, 512], f32, tag="ut")
                          nc.vector.tensor_tensor(ut[0:32, :], tn[64:96, :], smat[64:96, cs], ALU.mult)
                          nc.vector.tensor_tensor(ut[64:96, :], tn[0:32, :], smat[0:32, cs], ALU.mult)
                          nc.vector.tensor_tensor(
                              dstT[0:32, cs], dstT[0:32, cs].bitcast(f32), ut[0:32, :], ALU.add
                          )
                          nc.vector.tensor_tensor(
                              dstT[64:96, cs], dstT[64:96, cs].bitcast(f32), ut[64:96, :], ALU.add
                          )

              # ================= Phase 3: causal attention ======================
              with (
                  tc.tile_pool(name="s_ps", bufs=2, space=bass.MemorySpace.PSUM) as s_ps,
                  tc.tile_pool(name="y_ps", bufs=2, space=bass.MemorySpace.PSUM) as y_ps,
                  tc.tile_pool(name="r_ps", bufs=1, space=bass.MemorySpace.PSUM) as r_ps,
                  tc.tile_pool(name="b2_ps", bufs=1, space=bass.MemorySpace.PSUM) as b2_ps,
                  tc.tile_pool(name="pt", bufs=3) as pt_pool,
                  tc.tile_pool(name="att_sb", bufs=2) as att_sb,
              ):
                  for c in range(NCHUNK):
                      cs = bass.ts(c, 512)
                      n_s = 4 * (c + 1)
                      ps_y = y_ps.tile([128, 512], f32, tag="ps_y")
                      ps_r = r_ps.tile([1, 512], f32, tag="ps_r")
                      for pair in range(n_s // 2):
                          ps_S = s_ps.tile([128, 1024], f32, tag="ps_S")
                          for u in (0, 1):
                              i = 2 * pair + u
                              nc.tensor.matmul(
                                  ps_S[:, bass.ts(u, 512)],
                                  knT[:, bass.ts(i, 128)],
                                  qnT[:, cs],
                                  start=True, stop=True,
                              )
                              k_idx = i - 4 * c
                              if k_idx >= 0:
                                  w = 128 * (k_idx + 1)
                                  nc.vector.tensor_tensor(
                                      ps_S[:, 512 * u : 512 * u + w],
                                      ps_S[:, 512 * u : 512 * u + w],
                                      maskc[:, 512 * k_idx : 512 * k_idx + w],
                                      ALU.add,
                                  )
                          pT = pt_pool.tile([128, 1024], f32r, tag="pT")
                          nc.scalar.activation(pT[:], ps_S[:], FN.Exp)
                          for u in (0, 1):
                              i = 2 * pair + u
                              st, sp = i == 0, i == n_s - 1
                              nc.tensor.matmul(
                                  ps_y[:], v_sb[:, bass.ts(i, 128)], pT[:, bass.ts(u, 512)],
                                  start=st, stop=sp,
                              )
                              nc.tensor.matmul(
                                  ps_r[:], onc[:], pT[:, bass.ts(u, 512)],
                                  start=st, stop=sp,
                              )
                      # normalize y chunk by 1/rowsum and ship to a2a buffer
                      rrec = att_sb.tile([1, 512], f32, tag="rrec")
                      nc.vector.reciprocal(rrec[:], ps_r[:])
                      rrecr = att_sb.tile([1, 512], f32r, tag="rrecr")
                      nc.vector.tensor_copy(rrecr[:], rrec[:])
                      ps_br = b2_ps.tile([128, 512], f32, tag="ps_br")
                      nc.tensor.matmul(ps_br[:], onr[:], rrecr[:], start=True, stop=True)
                      brs = att_sb.tile([128, 512], f32, tag="brs")
                      nc.vector.tensor_copy(brs[:], ps_br[:])
                      yn = att_sb.tile([128, 512], f32r, tag="yn")
                      nc.vector.tensor_tensor(yn[:], ps_y[:], brs[:], ALU.mult)
                      nc.scalar.dma_start(a2a_in[128 * c : 128 * (c + 1), :], yn[:])

              nc.gpsimd.collective_compute(
                  "AllToAll",
                  ALU.bypass,
                  replica_groups=[list(range(N_CORES))],
                  ins=[a2a_in[:].opt()],
                  outs=[a2a_out[:].opt()],
              )

              # ================= Phase 4: output projection =====================
              with (
                  tc.tile_pool(name="proj_sb", bufs=1) as proj_sb,
                  tc.tile_pool(name="o_ps", bufs=2, space=bass.MemorySpace.PSUM) as o_ps,
                  tc.tile_pool(name="outp", bufs=3) as outp,
              ):
                  pw = proj_sb.tile([128, 8 * DIM], f32r, tag="pw")
                  nc.sync.dma_start(pw[:], pw_d[:])
                  yT = proj_sb.tile([128, N_CORES * TSLICE], f32r, tag="yT")
                  nc.sync.dma_start(
                      yT[:].rearrange("p (h t) -> p h t", h=N_CORES),
                      a2a_out[:].rearrange("(h p) t -> p h t", p=128),
                  )
                  for m in range(4):
                      for dc in range(2):
                          ps_o = o_ps.tile([128, 512], f32, tag="ps_o")
                          for hh in range(8):
                              nc.tensor.matmul(
                                  ps_o[:],
                                  yT[:, hh * TSLICE + 128 * m : hh * TSLICE + 128 * (m + 1)],
                                  pw[:, hh * DIM + 512 * dc : hh * DIM + 512 * (dc + 1)],
                                  start=(hh == 0), stop=(hh == 7),
                              )
                          ob = outp.tile([128, 512], f32, tag="ob")
                          nc.vector.tensor_copy(ob[:], ps_o[:])
                          nc.scalar.dma_start(
                              out_d[128 * m : 128 * (m + 1), 512 * dc : 512 * (dc + 1)], ob[:]
                          )

    nc.finalize()
    return nc


_PROGRAM = None


def _get_program():
    global _PROGRAM
    if _PROGRAM is None:
        _PROGRAM = _build_program()
    return _PROGRAM


def _host_prep(x, ve, qkv_w, lambdas, proj_w):
    x = np.asarray(x, dtype=np.float32).reshape(T, DIM)
    ve = np.asarray(ve, dtype=np.float32).reshape(T, HDIM)
    qkv_w = np.asarray(qkv_w, dtype=np.float32)
    lam = np.asarray(lambdas, dtype=np.float32)
    proj_w = np.asarray(proj_w, dtype=np.float32)

    xt = _trunc22(x.T)                                     # [DIM, T]

    # rope tables
    nfreq = HEAD_DIM // 4
    ang = (1.0 / 1024.0) ** np.linspace(0.0, 1.0, nfreq, dtype=np.float32)
    theta = np.arange(T, dtype=np.float32)[:, None] * ang[None, :]     # [T, 32]
    cosT = np.cos(theta).T.astype(np.float32)              # [32, T]
    sinT = np.sin(theta).T.astype(np.float32)
    cmat = np.empty((128, T), np.float32)
    smat = np.empty((128, T), np.float32)
    cmat[0:32] = cosT
    cmat[32:64] = 1.0
    cmat[64:96] = cosT
    cmat[96:128] = 1.0
    smat[0:32] = -sinT
    smat[32:64] = 0.0
    smat[64:96] = sinT
    smat[96:128] = 0.0

    # causal masks for the 4 diagonal sub-positions
    maskc = np.zeros((128, 2048), np.float32)
    tri = np.where(
        np.arange(128)[:, None] > np.arange(128)[None, :], MASK_NEG, 0.0
    ).astype(np.float32)
    for k in range(4):
        maskc[:, 512 * k : 512 * k + 128 * k] = MASK_NEG
        maskc[:, 512 * k + 128 * k : 512 * k + 128 * (k + 1)] = tri

    ones_col = _trunc22(np.ones((128, 1), np.float32))
    ones_row = _trunc22(np.ones((1, 128), np.float32))
    ident = np.eye(128, dtype=np.float32)

    in_maps = []
    for h in range(N_CORES):
        hs = slice(128 * h, 128 * (h + 1))
        # weight layout: w[p, dt*128 + m] = W[m, dt*128 + p]
        Wq = qkv_w[0, hs, :]                                # [128, DIM]
        Wk = qkv_w[1, hs, :]
        Wv = qkv_w[2, hs, :] * lam[0]
        def wlay(W):
            # [m, (dt p)] -> [p, (dt m)]
            a = W.reshape(128, 8, 128)                      # [m, dt, p]
            return _trunc22(np.ascontiguousarray(a.transpose(2, 1, 0).reshape(128, DIM)))
        # vew[p, i*128 + c] = lam1 * ve[i*128 + p, h*128 + c]
        veh = (ve[:, hs] * lam[1]).reshape(NTT, 128, 128)   # [i, p, c]
        vew = np.ascontiguousarray(veh.transpose(1, 0, 2).reshape(128, T))
        # pw[p, n*DIM + D] = proj_w[D, 128n + p]
        pwh = proj_w.T.reshape(8, 128, DIM)                 # [n, e_p, D]
        pw = _trunc22(np.ascontiguousarray(pwh.transpose(1, 0, 2).reshape(128, 8 * DIM)))
        in_maps.append(
            {
                "xt": xt,
                "wq": wlay(Wq),
                "wk": wlay(Wk),
                "wv": wlay(Wv),
                "vew": vew.astype(np.float32),
                "cmat": cmat,
                "smat": smat,
                "maskc": maskc,
                "pw": pw,
                "ones_col": ones_col,
                "ones_row": ones_row,
                "ident": ident,
            }
        )
    return in_maps


def kernel(x, ve, qkv_w, lambdas, proj_w):
    in_maps = _host_prep(x, ve, qkv_w, lambdas, proj_w)
    nc = _get_program()
    res = run_bass_kernel_spmd(nc, in_maps, list(range(N_CORES)))
    out = np.concatenate([res.results[c]["out"] for c in range(N_CORES)], axis=0)
    return out.reshape(B, T, DIM).astype(np.float32)


# ---------------------------------------------------------------------------
# Timing support (test.py only): run the program with device-resident inputs
# so repeated executions measure device time, and difference two repeat
# factors to cancel dispatch overhead.
# ---------------------------------------------------------------------------

def make_runner(in_maps, repeat=1):
    import jax
    from jax.sharding import Mesh, PartitionSpec, NamedSharding
    from jax.experimental.shard_map import shard_map
    from concourse import bass2jax
    from concourse.bass2jax import _bass_exec_p, partition_id_tensor

    bass2jax.install_neuronx_cc_hook()
    nc = _build_program(repeat)

    in_names, out_names, out_avals, zero_outs = [], [], [], []
    partition_name = nc.partition_id_tensor.name if nc.partition_id_tensor else None
    for alloc in nc.m.functions[0].allocations:
        if not isinstance(alloc, mybir.MemoryLocationSet):
            continue
        name = alloc.memorylocations[0].name
        if alloc.kind == "ExternalInput":
            if name != partition_name:
                in_names.append(name)
        elif alloc.kind == "ExternalOutput":
            out_names.append(name)
            shape = tuple(alloc.tensor_shape)
            dtype = mybir.dt.np(alloc.dtype)
            out_avals.append(jax.core.ShapedArray(shape, dtype))
            zero_outs.append(np.zeros(shape, dtype))
    n_params = len(in_names)
    n_outs = len(out_avals)
    all_in_names = list(in_names) + out_names
    if partition_name is not None:
        all_in_names.append(partition_name)
    donate = tuple(range(n_params, n_params + n_outs))

    def _body(*args):
        operands = list(args)
        if partition_name is not None:
            operands.append(partition_id_tensor())
        outs = _bass_exec_p.bind(
            *operands,
            out_avals=tuple(out_avals),
            in_names=tuple(all_in_names),
            out_names=tuple(out_names),
            lowering_input_output_aliases=(),
            sim_require_finite=True,
            sim_require_nnan=True,
            nc=nc,
        )
        return tuple(outs)

    devices = jax.devices()[:N_CORES]
    mesh = Mesh(np.asarray(devices), ("core",))
    in_specs = (PartitionSpec("core"),) * (n_params + n_outs)
    out_specs = (PartitionSpec("core"),) * n_outs
    fn = jax.jit(
        shard_map(_body, mesh=mesh, in_specs=in_specs, out_specs=out_specs, check_rep=False),
        donate_argnums=donate,
        keep_unused=True,
    )
    sh = NamedSharding(mesh, PartitionSpec("core"))
    concat_in = [
        jax.device_put(
            np.concatenate([np.asarray(in_maps[c][nm]) for c in range(N_CORES)], axis=0), sh
        )
        for nm in in_names
    ]
    zero_glob = [np.zeros((N_CORES * z.shape[0], *z.shape[1:]), z.dtype) for z in zero_outs]

    def run_once():
        zs = [jax.device_put(z, sh) for z in zero_glob]
        outs = fn(*concat_in, *zs)
        for o in outs:
            o.block_until_ready()
        return outs

    return run_once



# revision 21
# speedup vs baseline: 4.0864x; 4.0864x over previous
"""Distributed Trainium2 Bass kernel v2 for nn_CausalSelfAttention (8 cores).

Per core = one head (head_dim 128). All heavy matmul operands bf16
(1 cyc/row at any width); f32 PSUM accumulation.

Key structure vs v1:
  - qkv: q,k computed in [d,t] ([128,1024] q|k psum); v computed DIRECTLY in
    [t,d] tiles (xt as stationary, wv^T as moving) - no transpose pass.
  - rms-norm factors: colsum of squares via (1/128)-column matmul, then
    DVE reciprocal + ACT Sqrt (avoids Ln/Exp act-table thrash);
    q-norm applied via 0.12-row broadcast matmul + in-place DVE mult;
    k-norm FOLDED INTO the exp activation's per-partition scale vector.
  - rope: q and k processed together on [128,1024] bf16 tiles (DVE 2x mode).
  - attention: S^T=[s,t] per s-tile (diag-trimmed widths); exp per s-tile with
    scale=rsq_k column; y-matmul uses pT as stationary and [v|ones] (129 cols)
    as moving -> y in [t,d] PLUS softmax denominator column in one pass
    (kills the separate row-sum matmul). Normalize via per-partition
    tensor_scalar, transpose y tiles to [d,t], AllToAll, then projection.
  - norm/aux chain runs one chunk behind qkv matmuls (software pipeline).
"""

import sys

sys.path.insert(0, "/opt/trn_rl_repo")

import numpy as np
import ml_dtypes
import concourse.bass as bass
import concourse.bacc as bacc
import concourse.mybir as mybir
from concourse import tile
from concourse.bass_utils import run_bass_kernel_spmd

N_CORES = 8
B, T, DIM = 1, 4096, 1024
NUM_HEADS, HEAD_DIM = 8, 128
HDIM = NUM_HEADS * HEAD_DIM
SCALE = 0.12
NCHUNK = T // 512           # 8 t-chunks of 512
NTT = T // 128              # 32 t-tiles / s-tiles of 128
TSLICE = T // N_CORES       # 512 output rows per core

f32 = mybir.dt.float32
f32r = mybir.dt.float32r
bf16 = mybir.dt.bfloat16
FN = mybir.ActivationFunctionType
ALU = mybir.AluOpType
MASK_NEG = -30000.0

BF16 = ml_dtypes.bfloat16

ALL_PHASES = ("qkv", "att", "a2a", "proj")


def _trunc22(a):
    b = np.ascontiguousarray(a, dtype=np.float32).copy()
    b.view(np.uint32)[...] &= 0xFFFFFC00
    return b


def _register_const(nc, value, dtype=f32):
    if (dtype, value) in nc.const_aps.aps:
        return
    t = nc.alloc_sbuf_tensor(f"const-{dtype.name}-{value}", [128, 1], dtype)
    nc.gpsimd.memset(t.ap(), value)
    nc.const_aps.aps[(dtype, value)] = t.ap()


def _build_program(repeat=1, phases=ALL_PHASES, barrier=False):
    nc = bacc.Bacc(num_devices=N_CORES)
    _register_const(nc, 0.0)
    nc.all_engine_barrier()

    # ---- DRAM parameters ----
    xt_d = nc.declare_dram_parameter("xt", [NCHUNK, 128, 8 * 512], bf16, isOutput=False)
    wq_d = nc.declare_dram_parameter("wq", [128, DIM], bf16, isOutput=False)
    wk_d = nc.declare_dram_parameter("wk", [128, DIM], bf16, isOutput=False)
    wvT_d = nc.declare_dram_parameter("wvT", [128, 8 * 128], bf16, isOutput=False)
    vew_d = nc.declare_dram_parameter("vew", [128, NTT * 128], bf16, isOutput=False)
    cm2_d = nc.declare_dram_parameter("cmat2", [128, T], bf16, isOutput=False)
    sm2_d = nc.declare_dram_parameter("smat2", [128, T], bf16, isOutput=False)
    tri_d = nc.declare_dram_parameter("tri", [128, 128], f32, isOutput=False)
    trib_d = nc.declare_dram_parameter("trib", [128, 128], bf16, isOutput=False)
    idb_d = nc.declare_dram_parameter("identb", [128, 128], bf16, isOutput=False)
    id_d = nc.declare_dram_parameter("ident", [128, 128], f32, isOutput=False)
    pw_d = nc.declare_dram_parameter("pw", [128, 8 * DIM], bf16, isOutput=False)
    invd_d = nc.declare_dram_parameter("invd", [128, 2], f32r, isOutput=False)
    onrq_d = nc.declare_dram_parameter("onrq", [1, 128], f32r, isOutput=False)
    out_d = nc.declare_dram_parameter("out", [TSLICE, DIM], f32, isOutput=True)

    for _rep in range(repeat):
      with tile.TileContext(nc, num_cores=N_CORES) as tc:
        with (
            tc.tile_pool(name="persist", bufs=1) as persist,
            tc.tile_pool(name="dram", bufs=1, space="DRAM") as dram,
        ):
            qknT = persist.tile([128, 2 * T], bf16, tag="qknT")
            v_sb = persist.tile([128, NTT, 129], bf16, tag="v_sb")
            tri = persist.tile([128, 128], f32, tag="tri")
            trib = persist.tile([128, 128], bf16, tag="trib")
            identb = persist.tile([128, 128], bf16, tag="identb")
            ident = persist.tile([128, 128], f32, tag="ident")
            rsqk_cols = persist.tile([128, NTT], f32, tag="rsqk_cols")
            invd = persist.tile([128, 2], f32r, tag="invd")      # 1/128 (x2 cols)
            onrq = persist.tile([1, 128], f32r, tag="onrq")      # 0.12

            nc.gpsimd.dma_start(tri[:], tri_d[:])
            nc.gpsimd.dma_start(trib[:], trib_d[:])
            nc.gpsimd.dma_start(identb[:], idb_d[:])
            nc.gpsimd.dma_start(ident[:], id_d[:])
            nc.gpsimd.dma_start(invd[:], invd_d[:])
            nc.gpsimd.dma_start(onrq[:], onrq_d[:])

            a2a_in = dram.tile([N_CORES * 128, TSLICE], bf16, tag="a2a_in")
            a2a_out = dram.tile([N_CORES * 128, TSLICE], bf16, tag="a2a_out")
            pw = persist.tile([128, 8 * DIM], bf16, tag="pw")

            # ================ Phase 1: qkv, norm factors, rope ================
            with (
                tc.tile_pool(name="wpool", bufs=1) as wpool,
                tc.tile_pool(name="ropec", bufs=1) as ropec,
                tc.tile_pool(name="xt", bufs=3) as xt_pool,
                tc.tile_pool(name="qk_ps", bufs=1, space=bass.MemorySpace.PSUM) as qk_ps,
                tc.tile_pool(name="v_ps", bufs=2, space=bass.MemorySpace.PSUM) as v_ps,
                tc.tile_pool(name="row_ps", bufs=1, space=bass.MemorySpace.PSUM) as row_ps,
                tc.tile_pool(name="bc_ps", bufs=1, space=bass.MemorySpace.PSUM) as bc_ps,
                tc.tile_pool(name="colT_ps", bufs=1, space=bass.MemorySpace.PSUM) as colT_ps,
                tc.tile_pool(name="xqk", bufs=3) as xqk_pool,
                tc.tile_pool(name="sqc", bufs=3) as sqc_pool,
                tc.tile_pool(name="rows", bufs=2) as rows,
                tc.tile_pool(name="tmps", bufs=2) as tmps,
            ):
                wq = wpool.tile([128, DIM], bf16, tag="wq")
                wk = wpool.tile([128, DIM], bf16, tag="wk")
                wvT = wpool.tile([128, 8 * 128], bf16, tag="wvT")
                cm2 = ropec.tile([128, T], bf16, tag="cm2")
                sm2 = ropec.tile([128, T], bf16, tag="sm2")
                vew = ropec.tile([128, NTT * 128], bf16, tag="vew")
                if "qkv" in phases:
                    # weights gate the first matmuls: scalar queue, first
                    nc.scalar.dma_start(wq[:], wq_d[:])
                    nc.scalar.dma_start(wk[:], wk_d[:])
                    nc.scalar.dma_start(wvT[:], wvT_d[:])
                    # chunk-0 slices up front; the rest stream per chunk
                    nc.gpsimd.dma_start(vew[:, 0:512], vew_d[:, 0:512])
                    nc.gpsimd.dma_start(cm2[:, 0:512], cm2_d[:, 0:512])
                    nc.gpsimd.dma_start(sm2[:, 0:512], sm2_d[:, 0:512])
                # ones column (col 128 of each v block): flat memset before
                # the first v-add overwrites cols 0:128 of each block
                nc.gpsimd.memset(v_sb[:], 1.0)

                state = {}
                for c in range(NCHUNK + 1 if "qkv" in phases else 0):
                    if c < NCHUNK:
                        if 0 < c:
                            cs4 = slice(512 * c, 512 * (c + 1))
                            nc.gpsimd.dma_start(vew[:, cs4], vew_d[:, cs4])
                            nc.gpsimd.dma_start(cm2[:, cs4], cm2_d[:, cs4])
                            nc.gpsimd.dma_start(sm2[:, cs4], sm2_d[:, cs4])
                        xt_t = xt_pool.tile([128, 8, 512], bf16, tag="xt")
                        nc.sync.dma_start(xt_t[:], xt_d[c, :, :].rearrange("p (dt col) -> p dt col", dt=8))

                        ps_qk = qk_ps.tile([128, 1024], f32, tag="ps_qk")
                        for dt in range(8):
                            st, sp = dt == 0, dt == 7
                            nc.tensor.matmul(ps_qk[:, 0:512], wq[:, bass.ts(dt, 128)], xt_t[:, dt, :], start=st, stop=sp)
                            nc.tensor.matmul(ps_qk[:, 512:1024], wk[:, bass.ts(dt, 128)], xt_t[:, dt, :], start=st, stop=sp)
                        ps_v = v_ps.tile([128, 512], f32, tag="ps_v")
                        for j in range(4):
                            for dt in range(8):
                                nc.tensor.matmul(
                                    ps_v[:, bass.ts(j, 128)],
                                    xt_t[:, dt, 128 * j : 128 * (j + 1)],
                                    wvT[:, bass.ts(dt, 128)],
                                    start=(dt == 0), stop=(dt == 7),
                                )
                        # evacuate q|k (ACT copy) and square (ACT, single operand)
                        xqk = xqk_pool.tile([128, 1024], bf16, tag="xqk")
                        nc.scalar.copy(xqk[:], ps_qk[:])
                        sqc = sqc_pool.tile([128, 1024], f32r, tag="sqc")
                        nc.scalar.activation(sqc[:], ps_qk[:], FN.Square)
                        # v += ve, into [t,d] v_sb slots
                        for j in range(4):
                            i = 4 * c + j
                            nc.vector.tensor_tensor(
                                v_sb[:, i, 0:128], ps_v[:, bass.ts(j, 128)],
                                vew[:, bass.ts(i, 128)], ALU.add,
                            )
                        state[c] = (xqk, sqc)

                    if c > 0:
                        p = c - 1
                        xqk, sqc = state.pop(p)
                        # mean of squares (1/128 folded into lhs column)
                        ps_rq = row_ps.tile([1, 512], f32, tag="ps_rq")
                        nc.tensor.matmul(ps_rq[:], invd[:, 0:1], sqc[:, 0:512], start=True, stop=True)
                        mrq = rows.tile([1, 512], f32, tag="mrq")
                        nc.vector.reciprocal(mrq[:], ps_rq[:])
                        rsq_q = rows.tile([1, 512], f32r, tag="rsq_q")
                        nc.scalar.activation(rsq_q[:], mrq[:], FN.Sqrt)
                        # broadcast 0.12*rsq_q to 128 rows
                        ps_b = bc_ps.tile([128, 512], f32, tag="ps_b")
                        nc.tensor.matmul(ps_b[:], onrq[:], rsq_q[:], start=True, stop=True)
                        # k mean-of-squares directly in column layout
                        ps_ct = colT_ps.tile([128, 4, 2], f32, tag="ps_ct")
                        for j in range(4):
                            nc.tensor.matmul(
                                ps_ct[:, j, :],
                                sqc[:, 512 + 128 * j : 512 + 128 * (j + 1)],
                                invd[:], start=True, stop=True,
                            )
                        mcol = rows.tile([128, 4], f32, tag="mcol")
                        nc.vector.reciprocal(mcol[:], ps_ct[:, :, 0])
                        nc.scalar.activation(rsqk_cols[:, 4 * p : 4 * p + 4], mcol[:], FN.Sqrt)
                        # q-normalize in place (bf16)
                        nc.vector.tensor_tensor(xqk[:, 0:512], ps_b[:], xqk[:, 0:512], ALU.mult)
                        # rope: q and k halves share the cos/sin slice
                        pc = bass.ts(p, 512)
                        ut = tmps.tile([128, 1024], bf16, tag="ut")
                        for half in (0, 1):
                            hc = slice(1024 * p + 512 * half, 1024 * p + 512 * (half + 1))
                            xh = slice(512 * half, 512 * (half + 1))
                            nc.vector.tensor_tensor(qknT[:, hc], xqk[:, xh], cm2[:, pc], ALU.mult)
                            nc.vector.tensor_tensor(ut[0:32, xh], xqk[64:96, xh], sm2[64:96, pc], ALU.mult)
                            nc.vector.tensor_tensor(ut[64:96, xh], xqk[0:32, xh], sm2[0:32, pc], ALU.mult)
                            nc.vector.tensor_tensor(
                                qknT[0:32, hc], qknT[0:32, hc].bitcast(bf16), ut[0:32, xh], ALU.add
                            )
                            nc.vector.tensor_tensor(
                                qknT[64:96, hc], qknT[64:96, hc].bitcast(bf16), ut[64:96, xh], ALU.add
                            )

            # ================= Phase 3: causal attention ======================
            with (
                tc.tile_pool(name="s_ps", bufs=3, space=bass.MemorySpace.PSUM) as s_ps,
                tc.tile_pool(name="y_ps", bufs=1, space=bass.MemorySpace.PSUM) as y_ps,
                tc.tile_pool(name="tr_ps", bufs=1, space=bass.MemorySpace.PSUM) as tr_ps,
                tc.tile_pool(name="pt", bufs=4) as pt_pool,
                tc.tile_pool(name="att_sb", bufs=2) as att_sb,
            ):
                if "proj" in phases:
                    nc.scalar.dma_start(pw[:], pw_d[:])
                for c in range(NCHUNK if "att" in phases else 0):
                    # one PSUM bank per t-tile: interleaved accumulation groups
                    # must not share a 2KB zero region
                    yblk = []
                    for j in range(4):
                        ysj = y_ps.tile([128, 129], f32, tag=f"ys{j}", name=f"ys{j}")
                        yblk.append(ysj)
                    n_s = 4 * (c + 1)
                    rrec = att_sb.tile([128, 4], f32, tag="rrec")
                    yn = att_sb.tile([128, 512], f32, tag="yn")
                    ps_tr = tr_ps.tile([128, 512], f32, tag="ps_tr")
                    for sig in range(n_s):
                        ws = 128 * max(0, sig - 4 * c)
                        ps_S = s_ps.tile([128, 512], f32, tag="ps_S")
                        diag = sig >= 4 * c
                        if diag:
                            # seed the diagonal 128x128 block with the causal
                            # mask (I^T @ tri), then accumulate k.q on top
                            nc.tensor.matmul(
                                ps_S[:, ws : ws + 128], identb[:], trib[:],
                                start=True, stop=False,
                            )
                        nc.tensor.matmul(
                            ps_S[:, ws:512],
                            qknT[:, 1024 * (sig // 4) + 512 + 128 * (sig % 4) : 1024 * (sig // 4) + 512 + 128 * (sig % 4) + 128],
                            qknT[:, 1024 * c + ws : 1024 * c + 512],
                            start=not diag, stop=True,
                        )
                        pT = pt_pool.tile([128, 512], bf16, tag="pT")
                        nc.scalar.activation(
                            pT[:, ws:512], ps_S[:, ws:512], FN.Exp,
                            scale=rsqk_cols[:, sig : sig + 1],
                        )
                        for j in range(max(0, sig - 4 * c), 4):
                            nc.tensor.matmul(
                                yblk[j][:],
                                pT[:, 128 * j : 128 * (j + 1)],
                                v_sb[:, sig, :],
                                start=(sig == 0), stop=(sig == 4 * c + j),
                            )
                        # tail for any t-tile whose accumulation just closed:
                        # overlaps the remaining s-tiles' matmuls
                        for j in range(4):
                            if sig == 4 * c + j:
                                nc.vector.reciprocal(rrec[:, j : j + 1], yblk[j][:, 128:129])
                                nc.vector.tensor_scalar(
                                    yn[:, bass.ts(j, 128)], yblk[j][:, 0:128],
                                    rrec[:, j : j + 1], None, ALU.mult,
                                )
                                nc.tensor.transpose(ps_tr[:, bass.ts(j, 128)], yn[:, bass.ts(j, 128)], ident[:])
                    a2a_st = att_sb.tile([128, 512], bf16, tag="a2a_st")
                    nc.vector.tensor_copy(a2a_st[:], ps_tr[:])
                    nc.scalar.dma_start(a2a_in[128 * c : 128 * (c + 1), :], a2a_st[:])

            if "a2a" in phases:
                nc.gpsimd.collective_compute(
                    "AllToAll",
                    ALU.bypass,
                    replica_groups=[list(range(N_CORES))],
                    ins=[a2a_in[:].opt()],
                    outs=[a2a_out[:].opt()],
                )

            # ================= Phase 4: output projection =====================
            with (
                tc.tile_pool(name="proj_sb", bufs=1) as proj_sb,
                tc.tile_pool(name="o_ps", bufs=2, space=bass.MemorySpace.PSUM) as o_ps,
                tc.tile_pool(name="outp", bufs=3) as outp,
            ):
                yT = proj_sb.tile([128, N_CORES, TSLICE], bf16, tag="yT")
                if "proj" in phases:
                    nc.sync.dma_start(
                        yT[:],
                        a2a_out[:].rearrange("(h p) t -> p h t", p=128),
                    )
                for m in range(4 if "proj" in phases else 0):
                    for dc in range(2):
                        ps_o = o_ps.tile([128, 512], f32, tag="ps_o")
                        for hh in range(8):
                            nc.tensor.matmul(
                                ps_o[:],
                                yT[:, hh, 128 * m : 128 * (m + 1)],
                                pw[:, hh * DIM + 512 * dc : hh * DIM + 512 * (dc + 1)],
                                start=(hh == 0), stop=(hh == 7),
                            )
                        ob = outp.tile([128, 512], f32, tag="ob")
                        nc.vector.tensor_copy(ob[:], ps_o[:])
                        nc.scalar.dma_start(
                            out_d[128 * m : 128 * (m + 1), 512 * dc : 512 * (dc + 1)], ob[:]
                        )

        if barrier:
            nc.all_engine_barrier()

    nc.finalize()
    return nc


_PROGRAM = None


def _get_program():
    global _PROGRAM
    if _PROGRAM is None:
        _PROGRAM = _build_program()
    return _PROGRAM


def _host_prep(x, ve, qkv_w, lambdas, proj_w):
    x = np.asarray(x, dtype=np.float32).reshape(T, DIM)
    ve = np.asarray(ve, dtype=np.float32).reshape(T, HDIM)
    qkv_w = np.asarray(qkv_w, dtype=np.float32)
    lam = np.asarray(lambdas, dtype=np.float32)
    proj_w = np.asarray(proj_w, dtype=np.float32)

    # x^T staged per chunk, contiguous per partition:
    # xt[c, p, 512*dt + col] = x[512c + col, 128dt + p]
    xr = x.reshape(NCHUNK, 512, 8, 128)              # [c, col, dt, p]
    xt = np.ascontiguousarray(xr.transpose(0, 3, 2, 1).reshape(NCHUNK, 128, 8 * 512)).astype(BF16)

    # rope tables, duplicated for q|k per chunk: [128, 2T] col block
    # [1024c : 1024c+512] = chunk c, [1024c+512 : 1024c+1024] = chunk c again
    nfreq = HEAD_DIM // 4
    ang = (1.0 / 1024.0) ** np.linspace(0.0, 1.0, nfreq, dtype=np.float32)
    theta = np.arange(T, dtype=np.float32)[:, None] * ang[None, :]     # [T, 32]
    cosT = np.cos(theta).T.astype(np.float32)        # [32, T]
    sinT = np.sin(theta).T.astype(np.float32)
    cmat = np.empty((128, T), np.float32)
    smat = np.empty((128, T), np.float32)
    cmat[0:32] = cosT
    cmat[32:64] = 1.0
    cmat[64:96] = cosT
    cmat[96:128] = 1.0
    smat[0:32] = -sinT
    smat[32:64] = 0.0
    smat[64:96] = sinT
    smat[96:128] = 0.0
    cm2 = cmat.astype(BF16)
    sm2 = smat.astype(BF16)

    tri = np.where(
        np.arange(128)[:, None] > np.arange(128)[None, :], MASK_NEG, 0.0
    ).astype(np.float32)
    ident = np.eye(128, dtype=np.float32)

    in_maps = []
    for h in range(N_CORES):
        hs = slice(128 * h, 128 * (h + 1))
        Wq = qkv_w[0, hs, :]                         # [128, DIM]
        Wk = qkv_w[1, hs, :]
        Wv = qkv_w[2, hs, :] * lam[0]

        def wlay(W):
            # w[p, 128*dt + m] = W[m, 128dt + p]
            a = W.reshape(128, 8, 128)               # [m, dt, p]
            return np.ascontiguousarray(a.transpose(2, 1, 0).reshape(128, DIM)).astype(BF16)

        # wvT[p, 128*dt + dd] = Wv[dd, 128dt + p]
        wvT = wlay(Wv)

        # vew[p, 128*i + dd] = lam1 * ve[128i + p, 128h + dd]
        veh = (ve[:, hs] * lam[1]).reshape(NTT, 128, 128)   # [i, p, dd]
        vew = np.ascontiguousarray(veh.transpose(1, 0, 2).reshape(128, NTT * 128)).astype(BF16)

        # pw[p, 1024*n + e] = proj_w[e, 128n + p]
        pwh = proj_w.T.reshape(8, 128, DIM)          # [n, p, e]
        pw = np.ascontiguousarray(pwh.transpose(1, 0, 2).reshape(128, 8 * DIM)).astype(BF16)

        in_maps.append(
            {
                "xt": xt,
                "invd": _trunc22(np.full((128, 2), 1.0 / 128.0, np.float32)),
                "onrq": _trunc22(np.full((1, 128), SCALE, np.float32)),
                "wq": wlay(Wq),
                "wk": wlay(Wk),
                "wvT": wvT,
                "vew": vew,
                "cmat2": cm2,
                "smat2": sm2,
                "tri": tri,
                "ident": ident,
                "trib": tri.astype(BF16),
                "identb": ident.astype(BF16),
                "pw": pw,
            }
        )
    return in_maps


def kernel(x, ve, qkv_w, lambdas, proj_w):
    in_maps = _host_prep(x, ve, qkv_w, lambdas, proj_w)
    nc = _get_program()
    res = run_bass_kernel_spmd(nc, in_maps, list(range(N_CORES)))
    out = np.concatenate([res.results[c]["out"] for c in range(N_CORES)], axis=0)
    return out.reshape(B, T, DIM).astype(np.float32)


def make_runner(in_maps, repeat=1, **build_kwargs):
    import jax
    from jax.sharding import Mesh, PartitionSpec, NamedSharding
    from jax.experimental.shard_map import shard_map
    from concourse import bass2jax
    from concourse.bass2jax import _bass_exec_p, partition_id_tensor

    bass2jax.install_neuronx_cc_hook()
    nc = _build_program(repeat, **build_kwargs)

    in_names, out_names, out_avals, zero_outs = [], [], [], []
    partition_name = nc.partition_id_tensor.name if nc.partition_id_tensor else None
    for alloc in nc.m.functions[0].allocations:
        if not isinstance(alloc, mybir.MemoryLocationSet):
            continue
        name = alloc.memorylocations[0].name
        if alloc.kind == "ExternalInput":
            if name != partition_name:
                in_names.append(name)
        elif alloc.kind == "ExternalOutput":
            out_names.append(name)
            shape = tuple(alloc.tensor_shape)
            dtype = mybir.dt.np(alloc.dtype)
            out_avals.append(jax.core.ShapedArray(shape, dtype))
            zero_outs.append(np.zeros(shape, dtype))
    n_params = len(in_names)
    n_outs = len(out_avals)
    all_in_names = list(in_names) + out_names
    if partition_name is not None:
        all_in_names.append(partition_name)
    donate = tuple(range(n_params, n_params + n_outs))

    def _body(*args):
        operands = list(args)
        if partition_name is not None:
            operands.append(partition_id_tensor())
        outs = _bass_exec_p.bind(
            *operands,
            out_avals=tuple(out_avals),
            in_names=tuple(all_in_names),
            out_names=tuple(out_names),
            lowering_input_output_aliases=(),
            sim_require_finite=True,
            sim_require_nnan=True,
            nc=nc,
        )
        return tuple(outs)

    devices = jax.devices()[:N_CORES]
    mesh = Mesh(np.asarray(devices), ("core",))
    in_specs = (PartitionSpec("core"),) * (n_params + n_outs)
    out_specs = (PartitionSpec("core"),) * n_outs
    fn = jax.jit(
        shard_map(_body, mesh=mesh, in_specs=in_specs, out_specs=out_specs, check_rep=False),
        donate_argnums=donate,
        keep_unused=True,
    )
    sh = NamedSharding(mesh, PartitionSpec("core"))
    concat_in = [
        jax.device_put(
            np.concatenate([np.asarray(in_maps[c][nm]) for c in range(N_CORES)], axis=0), sh
        )
        for nm in in_names
    ]
    zero_glob = [np.zeros((N_CORES * z.shape[0], *z.shape[1:]), z.dtype) for z in zero_outs]

    def run_once():
        zs = [jax.device_put(z, sh) for z in zero_glob]
        outs = fn(*concat_in, *zs)
        # block_until_ready does not reliably fence device completion through
        # the axon tunnel; a (tiny) host readback of the result does.
        return np.asarray(outs[0][0, 0:8])

    return run_once
